# revision 31
# baseline (speedup 1.0000x reference)
"""Trainium2 Bass kernel for nn_ClassifierHeadMultiProposal.

Computation (reference.py): masked max-pool over words of `statement`
(N,A,Li,Lqa,D) -> residual MLP + two LN+linear heads -> masked temporal
scores -> per-(N,A) top-1 span via softmax outer-product argmax -> span /
global masked max-pools -> LN + linear classifier -> answer scores.

Sharding: data-parallel over batch N across 8 cores (2 batch rows = 10
(n,a) pairs per core). LN affine params are folded into the following
linear layers on the host (exact when g=1,b=0 as in setup_inputs; f64
fold otherwise).
"""

import sys

sys.path.insert(0, "/opt/trn_rl_repo")

from contextlib import ExitStack

import numpy as np

import concourse.bacc as bacc
import concourse.bass as bass  # noqa: F401  (dtype/AP helpers)
import concourse.tile as tile
from concourse import mybir
from concourse.bass_utils import run_bass_kernel_spmd

F32 = mybir.dt.float32
OP = mybir.AluOpType
AF = mybir.ActivationFunctionType
AX = mybir.AxisListType

NEG = -1e10
N, A, Li, Lqa, D = 16, 5, 32, 64, 1024
NCORES = 8
NPC = N // NCORES  # batch rows per core
PAIRS = NPC * A  # 10 (n,a) pairs per core
TOK = PAIRS * Li  # 320 tokens per core
GROUPS = [(0, 4), (4, 4), (8, 2)]  # (first pair, n pairs): 4*32=128 partitions
WSLAB = 2  # words per DMA slab
EPS = 1e-5


def _build_program(phases=9):
    nc = bacc.Bacc("TRN2", target_bir_lowering=False, debug=False)

    st_d = nc.dram_tensor("st", [PAIRS, Li, Lqa, D], F32, kind="ExternalInput").ap()
    sm_d = nc.dram_tensor("smask", [PAIRS, Li, Lqa], F32, kind="ExternalInput").ap()
    projw_d = nc.dram_tensor("projw", [D, D], F32, kind="ExternalInput").ap()
    projb_d = nc.dram_tensor("projb", [1, D], F32, kind="ExternalInput").ap()
    wstb_d = nc.dram_tensor("wstb", [128, D], F32, kind="ExternalInput").ap()
    wedb_d = nc.dram_tensor("wedb", [128, D], F32, kind="ExternalInput").ap()
    wcb_d = nc.dram_tensor("wcb", [PAIRS, 2 * D], F32, kind="ExternalInput").ap()
    ccol_d = nc.dram_tensor("ccol", [PAIRS, 1], F32, kind="ExternalInput").ap()
    ta_d = nc.dram_tensor("tacol", [TOK, 1], F32, kind="ExternalInput").ap()
    tbst_d = nc.dram_tensor("tbst", [TOK, 1], F32, kind="ExternalInput").ap()
    tbed_d = nc.dram_tensor("tbed", [TOK, 1], F32, kind="ExternalInput").ap()
    msk_d = nc.dram_tensor("mskcol", [TOK, 1], F32, kind="ExternalInput").ap()
    iota_d = nc.dram_tensor("iota", [PAIRS, Li], F32, kind="ExternalInput").ap()
    id_d = nc.dram_tensor("ident", [128, 128], F32, kind="ExternalInput").ap()

    tmp_o = nc.dram_tensor("temporal_o", [PAIRS, Li, 2], F32, kind="ExternalOutput").ap()
    ans_o = nc.dram_tensor("ans_o", [PAIRS], F32, kind="ExternalOutput").ap()

    with tile.TileContext(nc) as tc, ExitStack() as ctx:
        wpool = ctx.enter_context(tc.tile_pool(name="wts", bufs=1))
        spool = ctx.enter_context(tc.tile_pool(name="slab", bufs=3))
        apool = ctx.enter_context(tc.tile_pool(name="work", bufs=2))
        tpool = ctx.enter_context(tc.tile_pool(name="tiny", bufs=2))
        ppool = ctx.enter_context(tc.tile_pool(name="ps", bufs=2, space="PSUM"))
        mpool = ctx.enter_context(tc.tile_pool(name="mmps", bufs=2, space="PSUM"))

        # ---- persistent loads ----
        W = wpool.tile([128, 8, D], F32, name="W")
        nc.sync.dma_start(W, projw_d.rearrange("(k p) d -> p k d", p=128))
        projb_t = wpool.tile([1, D], F32, name="projb_t")
        nc.sync.dma_start(projb_t, projb_d)
        wstb_t = wpool.tile([128, D], F32, name="wstb_t")
        nc.sync.dma_start(wstb_t, wstb_d)
        wedb_t = wpool.tile([128, D], F32, name="wedb_t")
        nc.sync.dma_start(wedb_t, wedb_d)
        wcb_t = wpool.tile([PAIRS, 2 * D], F32, name="wcb_t")
        nc.sync.dma_start(wcb_t, wcb_d)
        ccol_t = wpool.tile([PAIRS, 1], F32, name="ccol_t")
        nc.scalar.dma_start(ccol_t, ccol_d)
        iota_t = wpool.tile([PAIRS, Li], F32, name="iota_t")
        nc.scalar.dma_start(iota_t, iota_d)
        id_t = wpool.tile([128, 128], F32, name="id_t")
        nc.sync.dma_start(id_t, id_d)
        ones_t = wpool.tile([1, 128], F32, name="ones_t")
        nc.vector.memset(ones_t, 1.0)

        def layernorm(x_ap, P, Dd, tag):
            s = tpool.tile([P, 1], F32, name=f"s_{tag}", tag=f"s_{tag}")
            nc.vector.reduce_sum(s, x_ap, axis=AX.X)
            negmu = tpool.tile([P, 1], F32, name=f"nmu_{tag}", tag=f"nmu_{tag}")
            nc.vector.tensor_scalar_mul(negmu, s, -1.0 / Dd)
            scr = apool.tile([P, Dd], F32, name=f"scr_{tag}", tag="scr")
            ssq = tpool.tile([P, 1], F32, name=f"ssq_{tag}", tag=f"ssq_{tag}")
            nc.vector.scalar_tensor_tensor(
                scr, x_ap, negmu, x_ap, OP.add, OP.mult, accum_out=ssq
            )
            tv = tpool.tile([P, 1], F32, name=f"tv_{tag}", tag=f"tv_{tag}")
            nc.vector.tensor_scalar(tv, ssq, 1.0 / Dd, EPS, OP.mult, OP.add)
            rec = tpool.tile([P, 1], F32, name=f"rec_{tag}", tag=f"rec_{tag}")
            nc.vector.reciprocal(rec, tv)
            rstd = tpool.tile([P, 1], F32, name=f"rstd_{tag}", tag=f"rstd_{tag}")
            nc.scalar.activation(rstd, rec, AF.Sqrt)
            ln = apool.tile([P, Dd], F32, name=f"ln_{tag}", tag="ln")
            nc.vector.tensor_scalar(ln, x_ap, negmu, rstd, OP.add, OP.mult)
            return ln

        for gi, (g0, ng) in enumerate(GROUPS):
            P = ng * Li
            t0 = g0 * Li

            # ---- phase A: masked max-pool over words ----
            mt = apool.tile([P, Lqa], F32, name="mt", tag="mt")
            nc.scalar.dma_start(mt, sm_d[g0 : g0 + ng].rearrange("a l w -> (a l) w"))
            ct = apool.tile([P, Lqa], F32, name="ct", tag="ct")
            nc.gpsimd.tensor_scalar(ct, mt, -NEG, NEG, OP.mult, OP.add)

            acc = apool.tile([P, WSLAB, D], F32, name="acc", tag="acc", bufs=1)
            accf = acc.rearrange("p w d -> p (w d)")
            for s in range(Lqa // WSLAB):
                if s == 0:
                    dst = acc
                else:
                    dst = spool.tile([P, WSLAB, D], F32, name="xs", tag="xs")
                nc.sync.dma_start(
                    dst,
                    st_d[g0 : g0 + ng, :, WSLAB * s : WSLAB * (s + 1), :].rearrange(
                        "a l w d -> (a l) w d"
                    ),
                )
                dstf = dst.rearrange("p w d -> p (w d)")
                for j in range(WSLAB):
                    w = WSLAB * s + j
                    nc.gpsimd.tensor_scalar(
                        dstf[:, j * D : (j + 1) * D],
                        dstf[:, j * D : (j + 1) * D],
                        mt[:, w : w + 1],
                        ct[:, w : w + 1],
                        OP.mult,
                        OP.add,
                    )
                if s > 0:
                    nc.vector.tensor_max(accf, accf, dstf)
            # fold WSLAB*D -> D
            ms = apool.tile([P, D], F32, name="ms", tag="ms")
            nc.vector.tensor_max(ms, accf[:, 0:D], accf[:, D : 2 * D])
            if phases < 2:
                continue

            # ---- phase B/C: LN + residual relu MLP ----
            ln1 = layernorm(ms, P, D, "1")
            lnT = []
            for c in range(8):
                pT = ppool.tile([128, P], F32, name="pT", tag="pT")
                nc.tensor.transpose(pT, ln1[:, c * 128 : (c + 1) * 128], id_t[0:P, 0:P])
                sT = apool.tile([128, P], F32, name=f"sT{c}", tag=f"sT{c}", bufs=1)
                nc.scalar.copy(sT, pT)
                lnT.append(sT)
            y = apool.tile([P, D], F32, name="y", tag="y")
            for h in range(2):
                pY = mpool.tile([P, 512], F32, name="pY", tag="pY")
                for k in range(8):
                    nc.tensor.matmul(
                        pY,
                        lhsT=lnT[k],
                        rhs=W[:, k, h * 512 : (h + 1) * 512],
                        start=(k == 0),
                        stop=False,
                    )
                nc.tensor.matmul(
                    pY,
                    lhsT=ones_t[:, 0:P],
                    rhs=projb_t[:, h * 512 : (h + 1) * 512],
                    start=False,
                    stop=True,
                )
                nc.scalar.activation(y[:, h * 512 : (h + 1) * 512], pY, AF.Relu)
            xt = apool.tile([P, D], F32, name="xt", tag="xt")
            nc.vector.tensor_add(xt, ms, y)
            if phases < 3:
                continue

            # ---- phase D: st/ed heads + temporal output ----
            ln2 = layernorm(xt, P, D, "2")
            scr_st = apool.tile([P, D], F32, name="scr_st", tag="scr")
            nc.vector.tensor_mul(scr_st, ln2, wstb_t[0:P])
            sst = tpool.tile([P, 1], F32, name="sst", tag="sst")
            nc.vector.reduce_sum(sst, scr_st, axis=AX.X)
            scr_ed = apool.tile([P, D], F32, name="scr_ed", tag="scr")
            nc.vector.tensor_mul(scr_ed, ln2, wedb_t[0:P])
            sed = tpool.tile([P, 1], F32, name="sed", tag="sed")
            nc.vector.reduce_sum(sed, scr_ed, axis=AX.X)
            ta_t = tpool.tile([P, 1], F32, name="ta_t", tag="ta_t")
            nc.scalar.dma_start(ta_t, ta_d[t0 : t0 + P])
            tbst_t = tpool.tile([P, 1], F32, name="tbst_t", tag="tbst_t")
            nc.scalar.dma_start(tbst_t, tbst_d[t0 : t0 + P])
            tbed_t = tpool.tile([P, 1], F32, name="tbed_t", tag="tbed_t")
            nc.scalar.dma_start(tbed_t, tbed_d[t0 : t0 + P])
            msk_t = tpool.tile([P, 1], F32, name="msk_t", tag="msk_t")
            nc.scalar.dma_start(msk_t, msk_d[t0 : t0 + P])

            tm = apool.tile([P, 2], F32, name="tm", tag="tm")
            nc.vector.tensor_scalar(tm[:, 0:1], sst, ta_t, tbst_t, OP.mult, OP.add)
            nc.vector.tensor_scalar(tm[:, 1:2], sed, ta_t, tbed_t, OP.mult, OP.add)
            nc.scalar.dma_start(
                tmp_o[g0 : g0 + ng].rearrange("a l c -> (a l) c"), tm
            )

            if phases < 4:
                continue

            # ---- phase E: softmax over Li + top-1 span (rows layout) ----
            def softmax_rows(src_col, tag):
                rows = tpool.tile([ng, Li], F32, name=f"rows_{tag}", tag=f"rows_{tag}")
                nc.scalar.dma_start(rows, src_col)
                rmax = tpool.tile([ng, 1], F32, name=f"rmax_{tag}", tag=f"rmax_{tag}")
                nc.vector.reduce_max(rmax, rows, axis=AX.X)
                nrm = tpool.tile([ng, 1], F32, name=f"nrm_{tag}", tag=f"nrm_{tag}")
                nc.vector.tensor_scalar_mul(nrm, rmax, -1.0)
                ex = tpool.tile([ng, Li], F32, name=f"ex_{tag}", tag=f"ex_{tag}")
                esum = tpool.tile([ng, 1], F32, name=f"esum_{tag}", tag=f"esum_{tag}")
                nc.scalar.activation(ex, rows, AF.Exp, bias=nrm, scale=1.0, accum_out=esum)
                rs = tpool.tile([ng, 1], F32, name=f"rs_{tag}", tag=f"rs_{tag}")
                nc.vector.reciprocal(rs, esum)
                p = tpool.tile([ng, Li], F32, name=f"p_{tag}", tag=f"p_{tag}")
                nc.vector.tensor_scalar(p, ex, rs, None, OP.mult)
                return p

            p_st = softmax_rows(tm[:, 0:1], "st")
            p_ed = softmax_rows(tm[:, 1:2], "ed")

            cmx = tpool.tile([ng, Li], F32, name="cmx", tag="cmx")
            nc.vector.tensor_tensor_scan(cmx, p_st, p_st, 0.0, OP.max, OP.max)
            q = tpool.tile([ng, Li], F32, name="q", tag="q")
            nc.vector.tensor_mul(q, cmx, p_ed)
            qm = tpool.tile([ng, 1], F32, name="qm", tag="qm")
            nc.vector.reduce_max(qm, q, axis=AX.X)
            ed_oh = tpool.tile([ng, Li], F32, name="ed_oh", tag="ed_oh")
            nc.vector.tensor_scalar(ed_oh, q, qm, None, OP.is_equal)
            scr_e = tpool.tile([ng, Li], F32, name="scr_e", tag="scr_e")
            nc.vector.tensor_mul(scr_e, cmx, ed_oh)
            cae = tpool.tile([ng, 1], F32, name="cae", tag="cae")
            nc.vector.reduce_sum(cae, scr_e, axis=AX.X)
            st_oh = tpool.tile([ng, Li], F32, name="st_oh", tag="st_oh")
            nc.vector.tensor_scalar(st_oh, p_st, cae, None, OP.is_equal)
            iota_g = iota_t[0:ng]
            scr_e2 = tpool.tile([ng, Li], F32, name="scr_e2", tag="scr_e2")
            nc.vector.tensor_mul(scr_e2, iota_g, st_oh)
            stv = tpool.tile([ng, 1], F32, name="stv", tag="stv")
            nc.vector.reduce_sum(stv, scr_e2, axis=AX.X)
            scr_e3 = tpool.tile([ng, Li], F32, name="scr_e3", tag="scr_e3")
            nc.vector.tensor_mul(scr_e3, iota_g, ed_oh)
            edv = tpool.tile([ng, 1], F32, name="edv", tag="edv")
            nc.vector.reduce_sum(edv, scr_e3, axis=AX.X)
            stm3 = tpool.tile([ng, 1], F32, name="stm3", tag="stm3")
            nc.vector.tensor_scalar(stm3, stv, -3.0, None, OP.add)
            edp3 = tpool.tile([ng, 1], F32, name="edp3", tag="edp3")
            nc.vector.tensor_scalar(edp3, edv, 3.0, None, OP.add)
            ge_t = tpool.tile([ng, Li], F32, name="ge_t", tag="ge_t")
            nc.vector.tensor_scalar(ge_t, iota_g, stm3, None, OP.is_ge)
            le_t = tpool.tile([ng, Li], F32, name="le_t", tag="le_t")
            nc.vector.tensor_scalar(le_t, iota_g, edp3, None, OP.is_le)
            span = tpool.tile([ng, Li], F32, name="span", tag="span")
            nc.vector.tensor_mul(span, ge_t, le_t)
            cmc = tpool.tile([P, 1], F32, name="cmc", tag="cmc")
            nc.scalar.dma_start(cmc, span)
            cmm = tpool.tile([P, 1], F32, name="cmm", tag="cmm")
            nc.vector.tensor_mul(cmm, cmc, msk_t)
            c2 = tpool.tile([P, 1], F32, name="c2", tag="c2")
            nc.gpsimd.tensor_scalar(c2, cmm, -NEG, NEG, OP.mult, OP.add)
            cmsk = tpool.tile([P, 1], F32, name="cmsk", tag="cmsk")
            nc.gpsimd.tensor_scalar(cmsk, msk_t, -NEG, NEG, OP.mult, OP.add)

            if phases < 5:
                continue

            # ---- phase F: loc/glob masked max over Li via PE transpose ----
            xg = apool.tile([P, D], F32, name="xg", tag="xg", bufs=1)
            nc.vector.tensor_scalar(xg, xt, msk_t, cmsk, OP.mult, OP.add)
            xm = apool.tile([P, D], F32, name="xm", tag="xm", bufs=1)
            nc.vector.tensor_scalar(xm, xt, cmm, c2, OP.mult, OP.add)
            mmg = apool.tile([ng, 2 * D], F32, name="mmg", tag="mmg")
            for src, base in ((xm, 0), (xg, D)):
                for c in range(8):
                    pT2 = ppool.tile([128, P], F32, name="pT2", tag="pT2")
                    nc.tensor.transpose(
                        pT2, src[:, c * 128 : (c + 1) * 128], id_t[0:P, 0:P]
                    )
                    red = tpool.tile([128, ng], F32, name="red", tag="red")
                    nc.vector.reduce_max(
                        red, pT2.rearrange("p (g l) -> p g l", l=Li), axis=AX.X
                    )
                    pG = ppool.tile([ng, 128], F32, name="pG", tag="pG")
                    nc.tensor.transpose(pG, red, id_t)
                    nc.scalar.copy(
                        mmg[:, base + c * 128 : base + (c + 1) * 128], pG
                    )

            if phases < 6:
                continue

            # ---- phase G: classifier head ----
            lnG = layernorm(mmg, ng, 2 * D, "G")
            scr_g = apool.tile([ng, 2 * D], F32, name="scr_g", tag="scr")
            nc.vector.tensor_mul(scr_g, lnG, wcb_t[0:ng])
            anst0 = tpool.tile([ng, 1], F32, name="anst0", tag="anst0")
            nc.vector.reduce_sum(anst0, scr_g, axis=AX.X)
            anst = tpool.tile([ng, 1], F32, name="anst", tag="anst")
            nc.vector.tensor_add(anst, anst0, ccol_t[0:ng])
            nc.scalar.dma_start(ans_o[g0 : g0 + ng], anst)

    nc.compile()
    return nc


def _host_prep(inputs):
    st = np.ascontiguousarray(np.asarray(inputs["statement"], dtype=np.float32))
    smask = np.ascontiguousarray(np.asarray(inputs["statement_mask"], dtype=np.float32))
    tsm = np.asarray(inputs["ts_labels_mask"], dtype=np.float64)  # (N, Li)

    pg = np.asarray(inputs["proj_g"], dtype=np.float64)
    pb = np.asarray(inputs["proj_b"], dtype=np.float64)
    pw = np.asarray(inputs["proj_w"], dtype=np.float64)
    pbias = np.asarray(inputs["proj_bias"], dtype=np.float64)
    projw = (pg[:, None] * pw).astype(np.float32)
    projb = (pbias + pb @ pw).astype(np.float32)[None, :]

    def fold_head(gk, bk, wk, bias_k):
        g = np.asarray(inputs[gk], dtype=np.float64)
        b = np.asarray(inputs[bk], dtype=np.float64)
        w = np.asarray(inputs[wk], dtype=np.float64)[:, 0]
        c = float(np.asarray(inputs[bias_k], dtype=np.float64)[0] + b @ w)
        return (g * w), c

    wst, cst = fold_head("st_g", "st_b", "st_w", "st_bias")
    wed, ced = fold_head("ed_g", "ed_b", "ed_w", "ed_bias")
    wcl, ccl = fold_head("cls_g", "cls_b", "cls_w", "cls_bias")

    wstb = np.tile(wst.astype(np.float32)[None, :], (128, 1))
    wedb = np.tile(wed.astype(np.float32)[None, :], (128, 1))
    wcb = np.tile(wcl.astype(np.float32)[None, :], (PAIRS, 1))
    ccol = np.full((PAIRS, 1), ccl, dtype=np.float32)

    # per-token (pair-major) columns; pair = (n_local, a), token = pair*Li + li
    msk_full = (smask.sum(axis=3) != 0).astype(np.float64)  # (N, A, Li)
    iota = np.tile(np.arange(Li, dtype=np.float32)[None, :], (PAIRS, 1))
    ident = np.eye(128, dtype=np.float32)

    shared = dict(
        projw=projw, projb=projb, wstb=wstb, wedb=wedb, wcb=wcb, ccol=ccol,
        iota=iota, ident=ident,
    )
    in_maps = []
    for c in range(NCORES):
        nlo = c * NPC
        tsm_c = np.repeat(tsm[nlo : nlo + NPC, None, :], A, axis=1)  # (NPC,A,Li)
        ta = tsm_c.reshape(TOK, 1)
        tbst = (cst * tsm_c + (1.0 - tsm_c) * NEG).reshape(TOK, 1)
        tbed = (ced * tsm_c + (1.0 - tsm_c) * NEG).reshape(TOK, 1)
        mskcol = msk_full[nlo : nlo + NPC].reshape(TOK, 1)
        m = dict(shared)
        m.update(
            st=st[nlo : nlo + NPC].reshape(PAIRS, Li, Lqa, D),
            smask=smask[nlo : nlo + NPC].reshape(PAIRS, Li, Lqa),
            tacol=ta.astype(np.float32),
            tbst=tbst.astype(np.float32),
            tbed=tbed.astype(np.float32),
            mskcol=mskcol.astype(np.float32),
        )
        in_maps.append(m)
    return in_maps


def kernel(**inputs):
    targets = np.asarray(inputs["targets"]).copy()
    in_maps = _host_prep(inputs)
    nc = _build_program()
    res = run_bass_kernel_spmd(nc, in_maps, list(range(NCORES))).results
    answer_scores = np.concatenate(
        [r["ans_o"].reshape(NPC, A) for r in res], axis=0
    )  # (N, A)
    temporal = np.concatenate(
        [r["temporal_o"].reshape(NPC, A, Li, 2) for r in res], axis=0
    )  # (N, A, Li, 2)
    return answer_scores, targets, temporal


# revision 41
# speedup vs baseline: 1.2475x; 1.2475x over previous
"""Trainium2 Bass kernel for nn_ClassifierHeadMultiProposal.

Computation (reference.py): masked max-pool over words of `statement`
(N,A,Li,Lqa,D) -> residual MLP + two LN+linear heads -> masked temporal
scores -> per-(N,A) top-1 span via softmax outer-product argmax -> span /
global masked max-pools -> LN + linear classifier -> answer scores.

Sharding: data-parallel over batch N across 8 cores (2 batch rows = 10
(n,a) pairs per core). LN affine params are folded into the following
linear layers on the host (exact when g=1,b=0 as in setup_inputs; f64
fold otherwise).
"""

import sys

sys.path.insert(0, "/opt/trn_rl_repo")

from contextlib import ExitStack

import numpy as np

import concourse.bacc as bacc
import concourse.bass as bass  # noqa: F401  (dtype/AP helpers)
import concourse.tile as tile
from concourse import mybir
from concourse.bass_utils import run_bass_kernel_spmd

F32 = mybir.dt.float32
OP = mybir.AluOpType
AF = mybir.ActivationFunctionType
AX = mybir.AxisListType

NEG = -1e10
N, A, Li, Lqa, D = 16, 5, 32, 64, 1024
NCORES = 8
NPC = N // NCORES  # batch rows per core
PAIRS = NPC * A  # 10 (n,a) pairs per core
TOK = PAIRS * Li  # 320 tokens per core
GROUPS = [(0, 4), (4, 4), (8, 2)]  # (first pair, n pairs): 4*32=128 partitions
WSLAB = 2  # words per DMA slab
EPS = 1e-5


def _build_program(phases=9):
    nc = bacc.Bacc("TRN2", target_bir_lowering=False, debug=False)

    st_d = nc.dram_tensor("st", [PAIRS, Li, Lqa, D], F32, kind="ExternalInput").ap()
    sm_d = nc.dram_tensor("smask", [PAIRS, Li, Lqa], F32, kind="ExternalInput").ap()
    projw_d = nc.dram_tensor("projw", [D, D], F32, kind="ExternalInput").ap()
    projb_d = nc.dram_tensor("projb", [1, D], F32, kind="ExternalInput").ap()
    wstb_d = nc.dram_tensor("wstb", [128, D], F32, kind="ExternalInput").ap()
    wedb_d = nc.dram_tensor("wedb", [128, D], F32, kind="ExternalInput").ap()
    wcb_d = nc.dram_tensor("wcb", [PAIRS, 2 * D], F32, kind="ExternalInput").ap()
    ccol_d = nc.dram_tensor("ccol", [PAIRS, 1], F32, kind="ExternalInput").ap()
    ta_d = nc.dram_tensor("tacol", [TOK, 1], F32, kind="ExternalInput").ap()
    tbst_d = nc.dram_tensor("tbst", [TOK, 1], F32, kind="ExternalInput").ap()
    tbed_d = nc.dram_tensor("tbed", [TOK, 1], F32, kind="ExternalInput").ap()
    msk_d = nc.dram_tensor("mskcol", [TOK, 1], F32, kind="ExternalInput").ap()
    iota_d = nc.dram_tensor("iota", [PAIRS, Li], F32, kind="ExternalInput").ap()
    id_d = nc.dram_tensor("ident", [128, 128], F32, kind="ExternalInput").ap()

    tmp_o = nc.dram_tensor("temporal_o", [PAIRS, Li, 2], F32, kind="ExternalOutput").ap()
    ans_o = nc.dram_tensor("ans_o", [PAIRS], F32, kind="ExternalOutput").ap()

    with tile.TileContext(nc) as tc, ExitStack() as ctx:
        wpool = ctx.enter_context(tc.tile_pool(name="wts", bufs=1))
        spool = ctx.enter_context(tc.tile_pool(name="slab", bufs=5))
        apool = ctx.enter_context(tc.tile_pool(name="work", bufs=2))
        tpool = ctx.enter_context(tc.tile_pool(name="tiny", bufs=2))
        gpool = ctx.enter_context(tc.tile_pool(name="big1", bufs=1))
        ppool = ctx.enter_context(tc.tile_pool(name="ps", bufs=2, space="PSUM"))
        mpool = ctx.enter_context(tc.tile_pool(name="mmps", bufs=2, space="PSUM"))

        # ---- persistent loads ----
        W = wpool.tile([128, 8, D], F32, name="W")
        nc.sync.dma_start(W, projw_d.rearrange("(k p) d -> p k d", p=128))
        projb_t = wpool.tile([1, D], F32, name="projb_t")
        nc.sync.dma_start(projb_t, projb_d)
        wstb_t = wpool.tile([128, D], F32, name="wstb_t")
        nc.sync.dma_start(wstb_t, wstb_d)
        wedb_t = wpool.tile([128, D], F32, name="wedb_t")
        nc.sync.dma_start(wedb_t, wedb_d)
        wcb_t = wpool.tile([PAIRS, 2 * D], F32, name="wcb_t")
        nc.sync.dma_start(wcb_t, wcb_d)
        ccol_t = wpool.tile([PAIRS, 1], F32, name="ccol_t")
        nc.scalar.dma_start(ccol_t, ccol_d)
        iota_t = wpool.tile([PAIRS, Li], F32, name="iota_t")
        nc.scalar.dma_start(iota_t, iota_d)
        id_t = wpool.tile([128, 128], F32, name="id_t")
        nc.sync.dma_start(id_t, id_d)
        ones_t = wpool.tile([1, 128], F32, name="ones_t")
        nc.vector.memset(ones_t, 1.0)

        def layernorm(x_ap, P, Dd, tag, big=False):
            # stats: sum on DVE; sum((x-mu)^2) via ACT Square(bias=-mu, accum)
            pool = gpool if big else apool
            bufs = 1 if big else None
            s = tpool.tile([P, 1], F32, name=f"s_{tag}", tag=f"s_{tag}")
            nc.vector.reduce_sum(s, x_ap, axis=AX.X)
            negmu = tpool.tile([P, 1], F32, name=f"nmu_{tag}", tag=f"nmu_{tag}")
            nc.vector.tensor_scalar_mul(negmu, s, -1.0 / Dd)
            scr = pool.tile([P, Dd], F32, name=f"scr_{tag}",
                            tag="scrg" if big else "scr", bufs=bufs)
            ssq = tpool.tile([P, 1], F32, name=f"ssq_{tag}", tag=f"ssq_{tag}")
            nc.scalar.activation(scr, x_ap, AF.Square, bias=negmu, accum_out=ssq)
            tv = tpool.tile([P, 1], F32, name=f"tv_{tag}", tag=f"tv_{tag}")
            nc.vector.tensor_scalar(tv, ssq, 1.0 / Dd, EPS, OP.mult, OP.add)
            rec = tpool.tile([P, 1], F32, name=f"rec_{tag}", tag=f"rec_{tag}")
            nc.vector.reciprocal(rec, tv)
            rstd = tpool.tile([P, 1], F32, name=f"rstd_{tag}", tag=f"rstd_{tag}")
            nc.scalar.activation(rstd, rec, AF.Sqrt)
            # ln = (x - mu)*rstd = x*rstd + (negmu*rstd) -> ACT Identity
            nmr = tpool.tile([P, 1], F32, name=f"nmr_{tag}", tag=f"nmr_{tag}")
            nc.vector.tensor_mul(nmr, negmu, rstd)
            ln = pool.tile([P, Dd], F32, name=f"ln_{tag}",
                           tag="lng" if big else "ln", bufs=bufs)
            nc.scalar.activation(ln, x_ap, AF.Identity, bias=nmr, scale=rstd)
            return ln

        for gi, (g0, ng) in enumerate(GROUPS):
            P = ng * Li
            t0 = g0 * Li

            # ---- phase A: masked max-pool over words ----
            mt = apool.tile([P, Lqa], F32, name="mt", tag="mt")
            nc.scalar.dma_start(mt, sm_d[g0 : g0 + ng].rearrange("a l w -> (a l) w"))
            ct = apool.tile([P, Lqa], F32, name="ct", tag="ct")
            nc.gpsimd.tensor_scalar(ct, mt, -NEG, NEG, OP.mult, OP.add)

            acc = apool.tile([P, WSLAB, D], F32, name="acc", tag="acc", bufs=1)
            accf = acc.rearrange("p w d -> p (w d)")
            for s in range(Lqa // WSLAB):
                if s == 0:
                    dst = acc
                else:
                    dst = spool.tile([P, WSLAB, D], F32, name="xs", tag="xs")
                nc.sync.dma_start(
                    dst,
                    st_d[g0 : g0 + ng, :, WSLAB * s : WSLAB * (s + 1), :].rearrange(
                        "a l w d -> (a l) w d"
                    ),
                )
                dstf = dst.rearrange("p w d -> p (w d)")
                for j in range(WSLAB):
                    w = WSLAB * s + j
                    sl = dstf[:, j * D : (j + 1) * D]
                    if s % 3 != 2:
                        nc.scalar.activation(
                            sl, sl, AF.Identity,
                            bias=ct[:, w : w + 1], scale=mt[:, w : w + 1],
                        )
                    else:
                        nc.gpsimd.tensor_scalar(
                            sl, sl, mt[:, w : w + 1], ct[:, w : w + 1],
                            OP.mult, OP.add,
                        )
                if s > 0:
                    nc.vector.tensor_max(accf, accf, dstf)
            # fold WSLAB*D -> D
            ms = apool.tile([P, D], F32, name="ms", tag="ms")
            nc.vector.tensor_max(ms, accf[:, 0:D], accf[:, D : 2 * D])
            if phases < 2:
                continue

            # ---- phase B/C: LN + residual relu MLP ----
            ln1 = layernorm(ms, P, D, "1")
            lnT = []
            for c in range(8):
                pT = ppool.tile([128, P], F32, name="pT", tag="pT")
                nc.tensor.transpose(pT, ln1[:, c * 128 : (c + 1) * 128], id_t[0:P, 0:P])
                sT = apool.tile([128, P], F32, name=f"sT{c}", tag=f"sT{c}", bufs=1)
                nc.scalar.copy(sT, pT)
                lnT.append(sT)
            y = apool.tile([P, D], F32, name="y", tag="y")
            for h in range(2):
                pY = mpool.tile([P, 512], F32, name="pY", tag="pY")
                for k in range(8):
                    nc.tensor.matmul(
                        pY,
                        lhsT=lnT[k],
                        rhs=W[:, k, h * 512 : (h + 1) * 512],
                        start=(k == 0),
                        stop=False,
                    )
                nc.tensor.matmul(
                    pY,
                    lhsT=ones_t[:, 0:P],
                    rhs=projb_t[:, h * 512 : (h + 1) * 512],
                    start=False,
                    stop=True,
                )
                nc.scalar.activation(y[:, h * 512 : (h + 1) * 512], pY, AF.Relu)
            xt = apool.tile([P, D], F32, name="xt", tag="xt")
            nc.vector.tensor_add(xt, ms, y)
            if phases < 3:
                continue

            # ---- phase D: st/ed heads + temporal output ----
            ln2 = layernorm(xt, P, D, "2")
            scr_st = apool.tile([P, D], F32, name="scr_st", tag="scr")
            nc.vector.tensor_mul(scr_st, ln2, wstb_t[0:P])
            sst = tpool.tile([P, 1], F32, name="sst", tag="sst")
            nc.vector.reduce_sum(sst, scr_st, axis=AX.X)
            scr_ed = apool.tile([P, D], F32, name="scr_ed", tag="scr")
            nc.vector.tensor_mul(scr_ed, ln2, wedb_t[0:P])
            sed = tpool.tile([P, 1], F32, name="sed", tag="sed")
            nc.vector.reduce_sum(sed, scr_ed, axis=AX.X)
            ta_t = tpool.tile([P, 1], F32, name="ta_t", tag="ta_t")
            nc.scalar.dma_start(ta_t, ta_d[t0 : t0 + P])
            tbst_t = tpool.tile([P, 1], F32, name="tbst_t", tag="tbst_t")
            nc.scalar.dma_start(tbst_t, tbst_d[t0 : t0 + P])
            tbed_t = tpool.tile([P, 1], F32, name="tbed_t", tag="tbed_t")
            nc.scalar.dma_start(tbed_t, tbed_d[t0 : t0 + P])
            msk_t = tpool.tile([P, 1], F32, name="msk_t", tag="msk_t")
            nc.scalar.dma_start(msk_t, msk_d[t0 : t0 + P])

            tm = apool.tile([P, 2], F32, name="tm", tag="tm")
            nc.scalar.activation(tm[:, 0:1], sst, AF.Identity, bias=tbst_t, scale=ta_t)
            nc.scalar.activation(tm[:, 1:2], sed, AF.Identity, bias=tbed_t, scale=ta_t)
            nc.scalar.dma_start(
                tmp_o[g0 : g0 + ng].rearrange("a l c -> (a l) c"), tm
            )

            if phases < 4:
                continue

            # ---- phase E: softmax over Li + top-1 span (rows layout) ----
            def softmax_rows(src_col, tag):
                rows = tpool.tile([ng, Li], F32, name=f"rows_{tag}", tag=f"rows_{tag}")
                nc.scalar.dma_start(rows, src_col)
                rmax = tpool.tile([ng, 1], F32, name=f"rmax_{tag}", tag=f"rmax_{tag}")
                nc.vector.reduce_max(rmax, rows, axis=AX.X)
                nrm = tpool.tile([ng, 1], F32, name=f"nrm_{tag}", tag=f"nrm_{tag}")
                nc.vector.tensor_scalar_mul(nrm, rmax, -1.0)
                ex = tpool.tile([ng, Li], F32, name=f"ex_{tag}", tag=f"ex_{tag}")
                esum = tpool.tile([ng, 1], F32, name=f"esum_{tag}", tag=f"esum_{tag}")
                nc.scalar.activation(ex, rows, AF.Exp, bias=nrm, scale=1.0, accum_out=esum)
                rs = tpool.tile([ng, 1], F32, name=f"rs_{tag}", tag=f"rs_{tag}")
                nc.vector.reciprocal(rs, esum)
                p = tpool.tile([ng, Li], F32, name=f"p_{tag}", tag=f"p_{tag}")
                nc.scalar.activation(p, ex, AF.Identity, bias=0.0, scale=rs)
                return p

            p_st = softmax_rows(tm[:, 0:1], "st")
            p_ed = softmax_rows(tm[:, 1:2], "ed")

            cmx = tpool.tile([ng, Li], F32, name="cmx", tag="cmx")
            nc.vector.tensor_tensor_scan(cmx, p_st, p_st, 0.0, OP.max, OP.max)
            q = tpool.tile([ng, Li], F32, name="q", tag="q")
            nc.vector.tensor_mul(q, cmx, p_ed)
            qm = tpool.tile([ng, 1], F32, name="qm", tag="qm")
            nc.vector.reduce_max(qm, q, axis=AX.X)
            ed_oh = tpool.tile([ng, Li], F32, name="ed_oh", tag="ed_oh")
            nc.gpsimd.tensor_scalar(ed_oh, q, qm, None, OP.is_equal)
            scr_e = tpool.tile([ng, Li], F32, name="scr_e", tag="scr_e")
            nc.vector.tensor_mul(scr_e, cmx, ed_oh)
            cae = tpool.tile([ng, 1], F32, name="cae", tag="cae")
            nc.vector.reduce_sum(cae, scr_e, axis=AX.X)
            st_oh = tpool.tile([ng, Li], F32, name="st_oh", tag="st_oh")
            nc.gpsimd.tensor_scalar(st_oh, p_st, cae, None, OP.is_equal)
            iota_g = iota_t[0:ng]
            scr_e2 = tpool.tile([ng, Li], F32, name="scr_e2", tag="scr_e2")
            nc.vector.tensor_mul(scr_e2, iota_g, st_oh)
            stv = tpool.tile([ng, 1], F32, name="stv", tag="stv")
            nc.vector.reduce_sum(stv, scr_e2, axis=AX.X)
            scr_e3 = tpool.tile([ng, Li], F32, name="scr_e3", tag="scr_e3")
            nc.vector.tensor_mul(scr_e3, iota_g, ed_oh)
            edv = tpool.tile([ng, 1], F32, name="edv", tag="edv")
            nc.vector.reduce_sum(edv, scr_e3, axis=AX.X)
            stm3 = tpool.tile([ng, 1], F32, name="stm3", tag="stm3")
            nc.gpsimd.tensor_scalar(stm3, stv, -3.0, None, OP.add)
            edp3 = tpool.tile([ng, 1], F32, name="edp3", tag="edp3")
            nc.gpsimd.tensor_scalar(edp3, edv, 3.0, None, OP.add)
            ge_t = tpool.tile([ng, Li], F32, name="ge_t", tag="ge_t")
            nc.gpsimd.tensor_scalar(ge_t, iota_g, stm3, None, OP.is_ge)
            le_t = tpool.tile([ng, Li], F32, name="le_t", tag="le_t")
            nc.gpsimd.tensor_scalar(le_t, iota_g, edp3, None, OP.is_le)
            span = tpool.tile([ng, Li], F32, name="span", tag="span")
            nc.vector.tensor_mul(span, ge_t, le_t)
            cmc = tpool.tile([P, 1], F32, name="cmc", tag="cmc")
            nc.scalar.dma_start(cmc, span)
            cmm = tpool.tile([P, 1], F32, name="cmm", tag="cmm")
            nc.vector.tensor_mul(cmm, cmc, msk_t)
            c2 = tpool.tile([P, 1], F32, name="c2", tag="c2")
            nc.gpsimd.tensor_scalar(c2, cmm, -NEG, NEG, OP.mult, OP.add)
            cmsk = tpool.tile([P, 1], F32, name="cmsk", tag="cmsk")
            nc.gpsimd.tensor_scalar(cmsk, msk_t, -NEG, NEG, OP.mult, OP.add)

            if phases < 5:
                continue

            # ---- phase F: loc/glob masked max over Li via PE transpose ----
            xg = apool.tile([P, D], F32, name="xg", tag="xg", bufs=1)
            nc.scalar.activation(xg, xt, AF.Identity, bias=cmsk, scale=msk_t)
            xm = apool.tile([P, D], F32, name="xm", tag="xm", bufs=1)
            nc.scalar.activation(xm, xt, AF.Identity, bias=c2, scale=cmm)
            mmg = apool.tile([ng, 2 * D], F32, name="mmg", tag="mmg")
            for src, base in ((xm, 0), (xg, D)):
                for c in range(8):
                    pT2 = ppool.tile([128, P], F32, name="pT2", tag="pT2")
                    nc.tensor.transpose(
                        pT2, src[:, c * 128 : (c + 1) * 128], id_t[0:P, 0:P]
                    )
                    red = tpool.tile([128, ng], F32, name="red", tag="red")
                    nc.vector.reduce_max(
                        red, pT2.rearrange("p (g l) -> p g l", l=Li), axis=AX.X
                    )
                    pG = ppool.tile([ng, 128], F32, name="pG", tag="pG")
                    nc.tensor.transpose(pG, red, id_t)
                    nc.scalar.copy(
                        mmg[:, base + c * 128 : base + (c + 1) * 128], pG
                    )

            if phases < 6:
                continue

            # ---- phase G: classifier head ----
            lnG = layernorm(mmg, ng, 2 * D, "G", big=True)
            scr_g = gpool.tile([ng, 2 * D], F32, name="scr_g", tag="scrg")
            nc.vector.tensor_mul(scr_g, lnG, wcb_t[0:ng])
            anst0 = tpool.tile([ng, 1], F32, name="anst0", tag="anst0")
            nc.vector.reduce_sum(anst0, scr_g, axis=AX.X)
            anst = tpool.tile([ng, 1], F32, name="anst", tag="anst")
            nc.vector.tensor_add(anst, anst0, ccol_t[0:ng])
            nc.scalar.dma_start(ans_o[g0 : g0 + ng], anst)

    nc.compile()
    return nc


def _host_prep(inputs):
    st = np.ascontiguousarray(np.asarray(inputs["statement"], dtype=np.float32))
    smask = np.ascontiguousarray(np.asarray(inputs["statement_mask"], dtype=np.float32))
    tsm = np.asarray(inputs["ts_labels_mask"], dtype=np.float64)  # (N, Li)

    pg = np.asarray(inputs["proj_g"], dtype=np.float64)
    pb = np.asarray(inputs["proj_b"], dtype=np.float64)
    pw = np.asarray(inputs["proj_w"], dtype=np.float64)
    pbias = np.asarray(inputs["proj_bias"], dtype=np.float64)
    projw = (pg[:, None] * pw).astype(np.float32)
    projb = (pbias + pb @ pw).astype(np.float32)[None, :]

    def fold_head(gk, bk, wk, bias_k):
        g = np.asarray(inputs[gk], dtype=np.float64)
        b = np.asarray(inputs[bk], dtype=np.float64)
        w = np.asarray(inputs[wk], dtype=np.float64)[:, 0]
        c = float(np.asarray(inputs[bias_k], dtype=np.float64)[0] + b @ w)
        return (g * w), c

    wst, cst = fold_head("st_g", "st_b", "st_w", "st_bias")
    wed, ced = fold_head("ed_g", "ed_b", "ed_w", "ed_bias")
    wcl, ccl = fold_head("cls_g", "cls_b", "cls_w", "cls_bias")

    wstb = np.tile(wst.astype(np.float32)[None, :], (128, 1))
    wedb = np.tile(wed.astype(np.float32)[None, :], (128, 1))
    wcb = np.tile(wcl.astype(np.float32)[None, :], (PAIRS, 1))
    ccol = np.full((PAIRS, 1), ccl, dtype=np.float32)

    # per-token (pair-major) columns; pair = (n_local, a), token = pair*Li + li
    msk_full = (smask.sum(axis=3) != 0).astype(np.float64)  # (N, A, Li)
    iota = np.tile(np.arange(Li, dtype=np.float32)[None, :], (PAIRS, 1))
    ident = np.eye(128, dtype=np.float32)

    shared = dict(
        projw=projw, projb=projb, wstb=wstb, wedb=wedb, wcb=wcb, ccol=ccol,
        iota=iota, ident=ident,
    )
    in_maps = []
    for c in range(NCORES):
        nlo = c * NPC
        tsm_c = np.repeat(tsm[nlo : nlo + NPC, None, :], A, axis=1)  # (NPC,A,Li)
        ta = tsm_c.reshape(TOK, 1)
        tbst = (cst * tsm_c + (1.0 - tsm_c) * NEG).reshape(TOK, 1)
        tbed = (ced * tsm_c + (1.0 - tsm_c) * NEG).reshape(TOK, 1)
        mskcol = msk_full[nlo : nlo + NPC].reshape(TOK, 1)
        m = dict(shared)
        m.update(
            st=st[nlo : nlo + NPC].reshape(PAIRS, Li, Lqa, D),
            smask=smask[nlo : nlo + NPC].reshape(PAIRS, Li, Lqa),
            tacol=ta.astype(np.float32),
            tbst=tbst.astype(np.float32),
            tbed=tbed.astype(np.float32),
            mskcol=mskcol.astype(np.float32),
        )
        in_maps.append(m)
    return in_maps


def kernel(**inputs):
    targets = np.asarray(inputs["targets"]).copy()
    in_maps = _host_prep(inputs)
    nc = _build_program()
    res = run_bass_kernel_spmd(nc, in_maps, list(range(NCORES))).results
    answer_scores = np.concatenate(
        [r["ans_o"].reshape(NPC, A) for r in res], axis=0
    )  # (N, A)
    temporal = np.concatenate(
        [r["temporal_o"].reshape(NPC, A, Li, 2) for r in res], axis=0
    )  # (N, A, Li, 2)
    return answer_scores, targets, temporal
